# revision 2
# baseline (speedup 1.0000x reference)
"""Trainium2 Bass kernel for nn_Expander (broadcast -> Conv3d(3->4) -> Conv3d(4->3)).

Math: the conv input is x (B,3) broadcast over all spatial positions, so the
whole network is an affine map per batch row:  out[b] = x[b] @ M + K0.
With two stacked kernel-3 SAME convs, out positions only depend on their
distance-from-edge class per axis: classes {0, 1, interior, n-2, n-1}.
So M/K0 compress to 3*5*5*5 = 375 distinct output columns.

Host side: fold (w1,b1,w2,b2) into W_aug (4, 375) via a 4-row numpy probe
(3 basis rows + zero row).  Columns are ordered (p, slot, ch, cw) with the
cd slot order [2,0,1,3,4] so the interior class comes first per channel.

Device side (per core, 128 batch rows), tuned for the ~368 GB/s per-core
HBM write roofline (19.3 MB out => ~52.4 us of pure DMA):
  1. matmul x_aug(128,4) @ W_aug cols 0:25 (p0 interior block) -> PSUM A,
     then cols 25:375 -> PSUM B.  [TensorE]
  2. w-expand (5 -> 28) reading PSUM directly, h-expand (5 -> 28) into
     8 d-slabs [cd0, cd1, I, I, I, I, cd3, cd4].  [DVE]
  3. 6 output DMAs, first one launched after only 6 vector copies; the
     d-axis 12x interior replication uses stride-0 (broadcast) DMA reads,
     so descriptors stay >= 3136B and instruction count stays tiny.
"""

import numpy as np

import concourse.bass as bass
import concourse.mybir as mybir
from concourse.tile import TileContext
from concourse.bass_utils import run_bass_kernel_spmd


def _ensure_axon_hooks_stub():
    """concourse imports antenv.axon_hooks when BASS_TRACE=1 under axon; the
    module is absent on this image.  Provide a no-op stub (profiling then
    degrades gracefully) unless a real one is already installed."""
    import sys, types

    try:
        import antenv.axon_hooks  # noqa: F401
    except ImportError:
        import antenv

        mod = types.ModuleType("antenv.axon_hooks")
        mod._hook = None
        mod.set_axon_ntff_profile_hook = lambda h: setattr(mod, "_hook", h)
        mod.get_axon_ntff_profile_hook = lambda: mod._hook
        sys.modules["antenv.axon_hooks"] = mod
        antenv.axon_hooks = mod


_ensure_axon_hooks_stub()


def _split_multi_waits(nc):
    """This container's walrus accepts at most ONE sync-wait (and update)
    command per instruction.  Tile can attach several (e.g. the kernel-tail
    Drain waits per outstanding semaphore; DMAs get cross-lane WAW waits).
    Hoist the extras onto injected same-engine NoOps: waits go on NoOps
    placed immediately BEFORE the instruction (waiting earlier on the same
    queue is equivalent), extra updates on NoOps AFTER it."""
    uid = [0]
    for f in nc.m.functions:
        for bb in f.blocks:
            out = []
            changed = False
            for inst in bb.instructions:
                si = getattr(inst, "sync_info", None)
                ow = list(si.on_wait) if si is not None and si.on_wait else []
                ou = list(si.on_update) if si is not None and si.on_update else []
                pre, post = [], []
                if len(ow) > 1 or len(ou) > 1:
                    def mknop(w=None, u=None):
                        uid[0] += 1
                        nop = mybir.InstNoOp(
                            name=f"{inst.name}-sw{uid[0]}",
                            opcode="NoOp",
                            engine=inst.engine,
                            debug=inst.debug,
                            ins=[],
                            outs=[],
                        )
                        nop.sync_info = mybir.SyncInfo(
                            on_wait=[w] if w else [], on_update=[u] if u else []
                        )
                        return nop

                    pre = [mknop(w=w) for w in ow[:-1]]
                    post = [mknop(u=u) for u in ou[1:]]
                    inst.sync_info = mybir.SyncInfo(
                        on_wait=ow[-1:], on_update=ou[:1]
                    )
                    changed = True
                out.extend(pre)
                out.append(inst)
                out.extend(post)
            if changed:
                bb.instructions = out


B, C, F, S = 1024, 3, 16, 28
P_OUT = 3
N_CORES = 8
BL = B // N_CORES  # 128 batch rows per core
NCLS = 5  # position classes per spatial axis
NJ = P_OUT * NCLS * NCLS * NCLS  # 375 distinct columns
NSL = 8  # d-slabs: [cd0, cd1, I, I, I, I, cd3, cd4]
SLOT_CD = [2, 0, 1, 3, 4]  # cd class per wexp slot (interior first)
F32 = mybir.dt.float32


def _conv3d_same(x, w):
    """x (B,Ci,D,H,W), w (Co,Ci,3,3,3) -> (B,Co,D,H,W), SAME padding."""
    Bp, Ci, D, H, W = x.shape
    xp = np.pad(x, ((0, 0), (0, 0), (1, 1), (1, 1), (1, 1)))
    out = np.zeros((Bp, w.shape[0], D, H, W), x.dtype)
    for kd in range(3):
        for kh in range(3):
            for kw in range(3):
                out += np.einsum(
                    "oc,bcdhw->bodhw",
                    w[:, :, kd, kh, kw],
                    xp[:, :, kd : kd + D, kh : kh + H, kw : kw + W],
                )
    return out


def _fold_weights(w1, b1, w2, b2):
    """Return W_aug (4, 375) float32: rows 0..2 = linear response to e_c at the
    5x5x5 class representatives, row 3 = constant term.  Column order is
    (p, slot, ch, cw) with slot = SLOT_CD order on the cd axis."""
    probe = np.zeros((4, C), np.float64)
    probe[:3] = np.eye(C)
    vp = np.broadcast_to(probe[:, :, None, None, None], (4, C, F, S, S)).astype(
        np.float64
    )
    y = _conv3d_same(vp, w1.astype(np.float64))
    y += b1.astype(np.float64)[None, :, None, None, None]
    y = _conv3d_same(y, w2.astype(np.float64))
    y += b2.astype(np.float64)[None, :, None, None, None]
    k0 = y[3]  # (3,16,28,28) constant part
    m = y[:3] - k0[None]  # (3,3,16,28,28) linear part

    dr = [0, 1, 2, F - 2, F - 1]
    hr = [0, 1, 2, S - 2, S - 1]
    mreps = m[:, :, dr][:, :, :, hr][:, :, :, :, hr]  # (3, 3, 5, 5, 5)
    kreps = k0[:, dr][:, :, hr][:, :, :, hr]  # (3, 5, 5, 5)
    mreps = mreps[:, :, SLOT_CD]  # cd axis -> slot order
    kreps = kreps[:, SLOT_CD]
    w_aug = np.empty((4, NJ), np.float64)
    w_aug[:3] = mreps.reshape(3, NJ)
    w_aug[3] = kreps.reshape(NJ)
    return np.ascontiguousarray(w_aug.astype(np.float32))


def _build_bass():
    nc = bass.Bass()
    # packed input: cols [0:BL] = x_aug^T (4,128), cols [BL:] = W_aug (4,375)
    xw = nc.dram_tensor("xw", [4, BL + NJ], F32, kind="ExternalInput")
    out = nc.dram_tensor("out", [BL, P_OUT, F, S, S], F32, kind="ExternalOutput")
    out_v = out[:].rearrange("b p d h w -> b p d (h w)")  # (128, 3, 16, 784)
    # interior frames 2:14 viewed as (j, k): frame = 2 + 4*j + k, slab = 2 + k
    mid = [
        out_v[:, p, 2:14, :].rearrange("b (j k) f -> b j k f", j=3)
        for p in range(P_OUT)
    ]

    J1 = NCLS * NCLS  # 25: p0 interior (slot 0) block

    with TileContext(nc) as tc:
        with (
            tc.tile_pool(name="pool", bufs=1) as pool,
            tc.tile_pool(name="psum", bufs=1, space="PSUM") as psum_pool,
        ):
            xw_sb = pool.tile([4, BL + NJ], F32)
            nc.sync.dma_start(out=xw_sb[:], in_=xw[:])

            ps_a = psum_pool.tile([BL, J1], F32)
            ps_b = psum_pool.tile([BL, NJ - J1], F32)
            nc.tensor.matmul(
                ps_a[:], xw_sb[:, :BL], xw_sb[:, BL : BL + J1],
                start=True, stop=True,
            )
            nc.tensor.matmul(
                ps_b[:], xw_sb[:, :BL], xw_sb[:, BL + J1 :],
                start=True, stop=True,
            )

            # wexp[b, p, slot, ch, w]: w-axis 5 -> 28;  dexp[b, p, slab, h, w]
            wexp = pool.tile([BL, P_OUT, NCLS, NCLS, S], F32)
            dexp = pool.tile([BL, P_OUT, NSL, S, S], F32)
            dv = dexp[:].rearrange("b p s h w -> b p s (h w)")  # (128, 3, 8, 784)

            def wexp_do(dst, src):
                """dst (BL, g, 5, 28) <- src (BL, g, 5, 5) w-expansion."""
                g = dst.shape[1]
                nc.vector.tensor_copy(
                    out=dst[:, :, :, 2 : S - 2],
                    in_=src[:, :, :, 2:3].to_broadcast((BL, g, NCLS, S - 4)),
                )
                nc.vector.tensor_copy(out=dst[:, :, :, 0:2], in_=src[:, :, :, 0:2])
                nc.vector.tensor_copy(
                    out=dst[:, :, :, S - 2 : S], in_=src[:, :, :, 3:5]
                )

            def hexp_interior(p, dsl, nsl):
                """dexp[:, p, dsl] (nsl slabs) <- wexp[:, p, 0] h-expansion."""
                dst = dexp[:, p, dsl]
                src = wexp[:, p, 0:1]  # (BL, 1, 5, 28)
                nc.vector.tensor_copy(
                    out=dst[:, :, 2 : S - 2, :],
                    in_=src[:, :, 2:3, :].to_broadcast((BL, nsl, S - 4, S)),
                )
                nc.vector.tensor_copy(
                    out=dst[:, :, 0:2, :],
                    in_=src[:, :, 0:2, :].to_broadcast((BL, nsl, 2, S)),
                )
                nc.vector.tensor_copy(
                    out=dst[:, :, S - 2 : S, :],
                    in_=src[:, :, 3:5, :].to_broadcast((BL, nsl, 2, S)),
                )

            def hexp_edges(p, dsl, ssl):
                """dexp[:, p, dsl] (2 slabs) <- wexp[:, p, ssl] (2 slots)."""
                dst = dexp[:, p, dsl]
                src = wexp[:, p, ssl]  # (BL, 2, 5, 28)
                nc.vector.tensor_copy(
                    out=dst[:, :, 2 : S - 2, :],
                    in_=src[:, :, 2:3, :].to_broadcast((BL, 2, S - 4, S)),
                )
                nc.vector.tensor_copy(out=dst[:, :, 0:2, :], in_=src[:, :, 0:2, :])
                nc.vector.tensor_copy(
                    out=dst[:, :, S - 2 : S, :], in_=src[:, :, 3:5, :]
                )

            def jbcast(src, j):
                """(BL, n, 784) -> (BL, j, n, 784) with stride-0 j axis."""
                n = src.shape[1]
                return src.rearrange("b (j s) f -> b j s f", j=1).to_broadcast(
                    (BL, j, n, 784)
                )

            # ---- p0 interior: shortest path to the first output DMA ----
            wexp_do(
                wexp[:, 0, 0:1],
                ps_a[:].rearrange("b (g ch cw) -> b g ch cw", g=1, ch=NCLS),
            )
            hexp_interior(0, slice(2, 3), 1)
            # frames {2, 6, 10} <- slab 2 (1.18 MB)
            nc.sync.dma_start(out=mid[0][:, :, 0:1, :], in_=jbcast(dv[:, 0, 2:3, :], 3))
            hexp_interior(0, slice(3, 6), 3)
            # frames {3,4,5, 7,8,9, 11,12,13} <- slabs 3:6 (3.53 MB)
            nc.sync.dma_start(out=mid[0][:, :, 1:4, :], in_=jbcast(dv[:, 0, 3:6, :], 3))

            # ---- remaining w-expansions (reading PSUM B directly) ----
            pb = ps_b[:].rearrange("b (g ch cw) -> b g ch cw", g=14, ch=NCLS)
            wexp_do(wexp[:, 0, 1:NCLS], pb[:, 0:4])
            wexp_do(
                wexp[:, 1:P_OUT].rearrange("b p n c w -> b (p n) c w"), pb[:, 4:14]
            )

            # ---- p1 / p2 interiors (4.82 MB DMAs, 12544B descriptors) ----
            hexp_interior(1, slice(2, 6), 4)
            nc.sync.dma_start(out=mid[1], in_=jbcast(dv[:, 1, 2:6, :], 3))
            hexp_interior(2, slice(2, 6), 4)
            nc.sync.dma_start(out=mid[2], in_=jbcast(dv[:, 2, 2:6, :], 3))

            # ---- edge slabs, then 2 merged cross-p edge DMAs (2.41 MB each) --
            for p in range(P_OUT):
                hexp_edges(p, slice(0, 2), slice(1, 3))
            nc.sync.dma_start(out=out_v[:, :, 0:2, :], in_=dv[:, :, 0:2, :])
            for p in range(P_OUT):
                hexp_edges(p, slice(6, 8), slice(3, 5))
            nc.sync.dma_start(
                out=out_v[:, :, F - 2 : F, :], in_=dv[:, :, 6:8, :]
            )
    _split_multi_waits(nc)
    return nc


_CACHE = {}


def kernel(x, w1, b1, w2, b2):
    x = np.ascontiguousarray(np.asarray(x, np.float32))
    w_aug = _fold_weights(
        np.asarray(w1, np.float64),
        np.asarray(b1, np.float64),
        np.asarray(w2, np.float64),
        np.asarray(b2, np.float64),
    )
    if "nc" not in _CACHE:
        _CACHE["nc"] = _build_bass()
    nc = _CACHE["nc"]

    # shard batch across cores; packed (4, 128+375): x_aug^T | W_aug
    in_maps = []
    for i in range(N_CORES):
        xs = x[i * BL : (i + 1) * BL]  # (128, 3)
        xa = np.concatenate([xs, np.ones((BL, 1), np.float32)], axis=1)  # (128,4)
        in_maps.append(
            {"xw": np.ascontiguousarray(np.concatenate([xa.T, w_aug], axis=1))}
        )
    res = run_bass_kernel_spmd(nc, in_maps, core_ids=list(range(N_CORES)))
    _CACHE["last_results"] = res  # exec_time_ns etc. when BASS_TRACE=1
    return np.concatenate([r["out"] for r in res.results], axis=0)


# revision 4
# speedup vs baseline: 1.0002x; 1.0002x over previous
"""Trainium2 Bass kernel for nn_Expander (broadcast -> Conv3d(3->4) -> Conv3d(4->3)).

Math: the conv input is x (B,3) broadcast over all spatial positions, so the
whole network is an affine map per batch row:  out[b] = x[b] @ M + K0.
With two stacked kernel-3 SAME convs, out positions only depend on their
distance-from-edge class per axis: classes {0, 1, interior, n-2, n-1}.
So M/K0 compress to 3*5*5*5 = 375 distinct output columns.

Host side: fold (w1,b1,w2,b2) into W_aug (4, 375) via a 4-row numpy probe
(3 basis rows + zero row).  Columns are ordered (p, slot, ch, cw) with the
cd slot order [2,0,1,3,4] so the interior class comes first per channel.

Device side (per core, 128 batch rows), tuned for the ~368 GB/s per-core
HBM write roofline (19.3 MB out => ~52.4 us of pure DMA):
  1. matmul x_aug(128,4) @ W_aug cols 0:25 (p0 interior block) -> PSUM A,
     then cols 25:375 -> PSUM B.  [TensorE]
  2. w-expand (5 -> 28) reading PSUM directly, h-expand (5 -> 28) into
     8 d-slabs [cd0, cd1, I, I, I, I, cd3, cd4].  [DVE]
  3. 6 output DMAs, first one launched after only 6 vector copies; the
     d-axis 12x interior replication uses stride-0 (broadcast) DMA reads,
     so descriptors stay >= 3136B and instruction count stays tiny.
"""

import numpy as np

import concourse.bass as bass
import concourse.mybir as mybir
from concourse.tile import TileContext
from concourse.bass_utils import run_bass_kernel_spmd


def _ensure_axon_hooks_stub():
    """concourse imports antenv.axon_hooks when BASS_TRACE=1 under axon; the
    module is absent on this image.  Provide a no-op stub (profiling then
    degrades gracefully) unless a real one is already installed."""
    import sys, types

    try:
        import antenv.axon_hooks  # noqa: F401
    except ImportError:
        import antenv

        mod = types.ModuleType("antenv.axon_hooks")
        mod._hook = None
        mod.set_axon_ntff_profile_hook = lambda h: setattr(mod, "_hook", h)
        mod.get_axon_ntff_profile_hook = lambda: mod._hook
        sys.modules["antenv.axon_hooks"] = mod
        antenv.axon_hooks = mod


_ensure_axon_hooks_stub()


def _split_multi_waits(nc):
    """This container's walrus accepts at most ONE sync-wait (and update)
    command per instruction.  Tile can attach several (e.g. the kernel-tail
    Drain waits per outstanding semaphore; DMAs get cross-lane WAW waits).
    Hoist the extras onto injected same-engine NoOps: waits go on NoOps
    placed immediately BEFORE the instruction (waiting earlier on the same
    queue is equivalent), extra updates on NoOps AFTER it."""
    uid = [0]
    for f in nc.m.functions:
        for bb in f.blocks:
            out = []
            changed = False
            for inst in bb.instructions:
                si = getattr(inst, "sync_info", None)
                ow = list(si.on_wait) if si is not None and si.on_wait else []
                ou = list(si.on_update) if si is not None and si.on_update else []
                pre, post = [], []
                if len(ow) > 1 or len(ou) > 1:
                    def mknop(w=None, u=None):
                        uid[0] += 1
                        nop = mybir.InstNoOp(
                            name=f"{inst.name}-sw{uid[0]}",
                            opcode="NoOp",
                            engine=inst.engine,
                            debug=inst.debug,
                            ins=[],
                            outs=[],
                        )
                        nop.sync_info = mybir.SyncInfo(
                            on_wait=[w] if w else [], on_update=[u] if u else []
                        )
                        return nop

                    pre = [mknop(w=w) for w in ow[:-1]]
                    post = [mknop(u=u) for u in ou[1:]]
                    inst.sync_info = mybir.SyncInfo(
                        on_wait=ow[-1:], on_update=ou[:1]
                    )
                    changed = True
                out.extend(pre)
                out.append(inst)
                out.extend(post)
            if changed:
                bb.instructions = out


B, C, F, S = 1024, 3, 16, 28
P_OUT = 3
N_CORES = 8
BL = B // N_CORES  # 128 batch rows per core
NCLS = 5  # position classes per spatial axis
NJ = P_OUT * NCLS * NCLS * NCLS  # 375 distinct columns
SLOT_CD = [2, 0, 1, 3, 4]  # cd class per wexp slot (interior first)
F32 = mybir.dt.float32


def _conv3d_same(x, w):
    """x (B,Ci,D,H,W), w (Co,Ci,3,3,3) -> (B,Co,D,H,W), SAME padding."""
    Bp, Ci, D, H, W = x.shape
    xp = np.pad(x, ((0, 0), (0, 0), (1, 1), (1, 1), (1, 1)))
    out = np.zeros((Bp, w.shape[0], D, H, W), x.dtype)
    for kd in range(3):
        for kh in range(3):
            for kw in range(3):
                out += np.einsum(
                    "oc,bcdhw->bodhw",
                    w[:, :, kd, kh, kw],
                    xp[:, :, kd : kd + D, kh : kh + H, kw : kw + W],
                )
    return out


def _fold_weights(w1, b1, w2, b2):
    """Return W_aug (4, 375) float32: rows 0..2 = linear response to e_c at the
    5x5x5 class representatives, row 3 = constant term.  Column order is
    (p, slot, ch, cw) with slot = SLOT_CD order on the cd axis."""
    probe = np.zeros((4, C), np.float64)
    probe[:3] = np.eye(C)
    vp = np.broadcast_to(probe[:, :, None, None, None], (4, C, F, S, S)).astype(
        np.float64
    )
    y = _conv3d_same(vp, w1.astype(np.float64))
    y += b1.astype(np.float64)[None, :, None, None, None]
    y = _conv3d_same(y, w2.astype(np.float64))
    y += b2.astype(np.float64)[None, :, None, None, None]
    k0 = y[3]  # (3,16,28,28) constant part
    m = y[:3] - k0[None]  # (3,3,16,28,28) linear part

    dr = [0, 1, 2, F - 2, F - 1]
    hr = [0, 1, 2, S - 2, S - 1]
    mreps = m[:, :, dr][:, :, :, hr][:, :, :, :, hr]  # (3, 3, 5, 5, 5)
    kreps = k0[:, dr][:, :, hr][:, :, :, hr]  # (3, 5, 5, 5)
    mreps = mreps[:, :, SLOT_CD]  # cd axis -> slot order
    kreps = kreps[:, SLOT_CD]
    w_aug = np.empty((4, NJ), np.float64)
    w_aug[:3] = mreps.reshape(3, NJ)
    w_aug[3] = kreps.reshape(NJ)
    return np.ascontiguousarray(w_aug.astype(np.float32))


def _build_bass():
    nc = bass.Bass()
    # packed input: cols [0:BL] = x_aug^T (4,128), cols [BL:] = W_aug (4,375)
    xw = nc.dram_tensor("xw", [4, BL + NJ], F32, kind="ExternalInput")
    out = nc.dram_tensor("out", [BL, P_OUT, F, S, S], F32, kind="ExternalOutput")
    out_v = out[:].rearrange("b p d h w -> b p d (h w)")  # (128, 3, 16, 784)

    J1 = NCLS * NCLS  # 25: p0 interior (slot 0) block

    with TileContext(nc) as tc:
        with (
            tc.tile_pool(name="pool", bufs=1) as pool,
            tc.tile_pool(name="psum", bufs=1, space="PSUM") as psum_pool,
        ):
            xw_sb = pool.tile([4, BL + NJ], F32)
            nc.sync.dma_start(out=xw_sb[:], in_=xw[:])

            ps_a = psum_pool.tile([BL, J1], F32)
            ps_b = psum_pool.tile([BL, NJ - J1], F32)
            nc.tensor.matmul(
                ps_a[:], xw_sb[:, :BL], xw_sb[:, BL : BL + J1],
                start=True, stop=True,
            )
            nc.tensor.matmul(
                ps_b[:], xw_sb[:, :BL], xw_sb[:, BL + J1 :],
                start=True, stop=True,
            )

            # wexp[b, p, slot, ch, w]: w-axis 5 -> 28
            # dexp[b, p, d, h, w]: fully materialized output frames
            wexp = pool.tile([BL, P_OUT, NCLS, NCLS, S], F32)
            dexp = pool.tile([BL, P_OUT, F, S, S], F32)
            dv = dexp[:].rearrange("b p d h w -> b p d (h w)")  # (128, 3, 16, 784)

            def wexp_do(dst, src):
                """dst (BL, g, 5, 28) <- src (BL, g, 5, 5) w-expansion."""
                g = dst.shape[1]
                nc.vector.tensor_copy(
                    out=dst[:, :, :, 2 : S - 2],
                    in_=src[:, :, :, 2:3].to_broadcast((BL, g, NCLS, S - 4)),
                )
                nc.vector.tensor_copy(out=dst[:, :, :, 0:2], in_=src[:, :, :, 0:2])
                nc.vector.tensor_copy(
                    out=dst[:, :, :, S - 2 : S], in_=src[:, :, :, 3:5]
                )

            def hexp_interior(p, dsl, nf):
                """dexp[:, p, dsl] (nf frames) <- wexp[:, p, 0] h-expansion."""
                dst = dexp[:, p, dsl]
                src = wexp[:, p, 0:1]  # (BL, 1, 5, 28)
                nc.vector.tensor_copy(
                    out=dst[:, :, 2 : S - 2, :],
                    in_=src[:, :, 2:3, :].to_broadcast((BL, nf, S - 4, S)),
                )
                nc.vector.tensor_copy(
                    out=dst[:, :, 0:2, :],
                    in_=src[:, :, 0:2, :].to_broadcast((BL, nf, 2, S)),
                )
                nc.vector.tensor_copy(
                    out=dst[:, :, S - 2 : S, :],
                    in_=src[:, :, 3:5, :].to_broadcast((BL, nf, 2, S)),
                )

            def hexp_edges(p, dsl, ssl):
                """dexp[:, p, dsl] (2 frames) <- wexp[:, p, ssl] (2 slots)."""
                dst = dexp[:, p, dsl]
                src = wexp[:, p, ssl]  # (BL, 2, 5, 28)
                nc.vector.tensor_copy(
                    out=dst[:, :, 2 : S - 2, :],
                    in_=src[:, :, 2:3, :].to_broadcast((BL, 2, S - 4, S)),
                )
                nc.vector.tensor_copy(out=dst[:, :, 0:2, :], in_=src[:, :, 0:2, :])
                nc.vector.tensor_copy(
                    out=dst[:, :, S - 2 : S, :], in_=src[:, :, 3:5, :]
                )

            # ---- p0 interior: shortest path to the first output DMA ----
            wexp_do(
                wexp[:, 0, 0:1],
                ps_a[:].rearrange("b (g ch cw) -> b g ch cw", g=1, ch=NCLS),
            )
            hexp_interior(0, slice(2, 3), 1)
            nc.sync.dma_start(out=out_v[:, 0, 2:3, :], in_=dv[:, 0, 2:3, :])
            hexp_interior(0, slice(3, 6), 3)
            nc.sync.dma_start(out=out_v[:, 0, 3:6, :], in_=dv[:, 0, 3:6, :])
            hexp_interior(0, slice(6, F - 2), 8)
            nc.sync.dma_start(out=out_v[:, 0, 6 : F - 2, :], in_=dv[:, 0, 6 : F - 2, :])

            # ---- remaining w-expansions (reading PSUM B directly) ----
            pb = ps_b[:].rearrange("b (g ch cw) -> b g ch cw", g=14, ch=NCLS)
            wexp_do(wexp[:, 0, 1:NCLS], pb[:, 0:4])
            wexp_do(
                wexp[:, 1:P_OUT].rearrange("b p n c w -> b (p n) c w"), pb[:, 4:14]
            )

            # ---- p1 + p2 interiors: one DMA, 37632B descriptors (9.63 MB) ----
            hexp_interior(1, slice(2, F - 2), F - 4)
            hexp_interior(2, slice(2, F - 2), F - 4)
            nc.sync.dma_start(
                out=out_v[:, 1:P_OUT, 2 : F - 2, :], in_=dv[:, 1:P_OUT, 2 : F - 2, :]
            )

            # ---- edge frames, then 2 merged cross-p edge DMAs (2.41 MB each) --
            for p in range(P_OUT):
                hexp_edges(p, slice(0, 2), slice(1, 3))
            nc.sync.dma_start(out=out_v[:, :, 0:2, :], in_=dv[:, :, 0:2, :])
            for p in range(P_OUT):
                hexp_edges(p, slice(F - 2, F), slice(3, 5))
            nc.sync.dma_start(
                out=out_v[:, :, F - 2 : F, :], in_=dv[:, :, F - 2 : F, :]
            )
    _split_multi_waits(nc)
    return nc


_CACHE = {}


def kernel(x, w1, b1, w2, b2):
    x = np.ascontiguousarray(np.asarray(x, np.float32))
    w_aug = _fold_weights(
        np.asarray(w1, np.float64),
        np.asarray(b1, np.float64),
        np.asarray(w2, np.float64),
        np.asarray(b2, np.float64),
    )
    if "nc" not in _CACHE:
        _CACHE["nc"] = _build_bass()
    nc = _CACHE["nc"]

    # shard batch across cores; packed (4, 128+375): x_aug^T | W_aug
    in_maps = []
    for i in range(N_CORES):
        xs = x[i * BL : (i + 1) * BL]  # (128, 3)
        xa = np.concatenate([xs, np.ones((BL, 1), np.float32)], axis=1)  # (128,4)
        in_maps.append(
            {"xw": np.ascontiguousarray(np.concatenate([xa.T, w_aug], axis=1))}
        )
    res = run_bass_kernel_spmd(nc, in_maps, core_ids=list(range(N_CORES)))
    _CACHE["last_results"] = res  # exec_time_ns etc. when BASS_TRACE=1
    return np.concatenate([r["out"] for r in res.results], axis=0)


# revision 6
# speedup vs baseline: 1.1166x; 1.1163x over previous
"""Trainium2 Bass kernel for nn_Expander (broadcast -> Conv3d(3->4) -> Conv3d(4->3)).

Math: the conv input is x (B,3) broadcast over all spatial positions, so the
whole network is an affine map per batch row:  out[b] = x[b] @ M + K0.
With two stacked kernel-3 SAME convs, out positions only depend on their
distance-from-edge class per axis: classes {0, 1, interior, n-2, n-1}.
So M/K0 compress to 3*5*5*5 = 375 distinct output columns.

Host side: fold (w1,b1,w2,b2) into W_aug (4, 375) via a 4-row numpy probe
(3 basis rows + zero row).  Columns are ordered (p, slot, ch, cw) with the
cd slot order [2,0,1,3,4] so the interior class comes first per channel.

Device side (per core, 128 batch rows), tuned for the ~368 GB/s per-core
HBM write roofline (19.3 MB out => ~52.4 us of pure DMA):
  1. matmul x_aug(128,4) @ W_aug cols 0:25 (p0 interior block) -> PSUM A,
     then cols 25:375 -> PSUM B.  [TensorE]
  2. w-expand (5 -> 28) reading PSUM directly, h-expand (5 -> 28) into
     8 d-slabs [cd0, cd1, I, I, I, I, cd3, cd4].  [DVE]
  3. 6 output DMAs, first one launched after only 6 vector copies; the
     d-axis 12x interior replication uses stride-0 (broadcast) DMA reads,
     so descriptors stay >= 3136B and instruction count stays tiny.
"""

import numpy as np

import concourse.bass as bass
import concourse.mybir as mybir
from concourse.tile import TileContext
from concourse.bass_utils import run_bass_kernel_spmd


def _ensure_axon_hooks_stub():
    """concourse imports antenv.axon_hooks when BASS_TRACE=1 under axon; the
    module is absent on this image.  Provide a no-op stub (profiling then
    degrades gracefully) unless a real one is already installed."""
    import sys, types

    try:
        import antenv.axon_hooks  # noqa: F401
    except ImportError:
        import antenv

        mod = types.ModuleType("antenv.axon_hooks")
        mod._hook = None
        mod.set_axon_ntff_profile_hook = lambda h: setattr(mod, "_hook", h)
        mod.get_axon_ntff_profile_hook = lambda: mod._hook
        sys.modules["antenv.axon_hooks"] = mod
        antenv.axon_hooks = mod


_ensure_axon_hooks_stub()


def _split_multi_waits(nc):
    """This container's walrus accepts at most ONE sync-wait (and update)
    command per instruction.  Tile can attach several (e.g. the kernel-tail
    Drain waits per outstanding semaphore; DMAs get cross-lane WAW waits).
    Hoist the extras onto injected same-engine NoOps: waits go on NoOps
    placed immediately BEFORE the instruction (waiting earlier on the same
    queue is equivalent), extra updates on NoOps AFTER it."""
    uid = [0]
    for f in nc.m.functions:
        for bb in f.blocks:
            out = []
            changed = False
            for inst in bb.instructions:
                si = getattr(inst, "sync_info", None)
                ow = list(si.on_wait) if si is not None and si.on_wait else []
                ou = list(si.on_update) if si is not None and si.on_update else []
                pre, post = [], []
                if len(ow) > 1 or len(ou) > 1:
                    def mknop(w=None, u=None):
                        uid[0] += 1
                        nop = mybir.InstNoOp(
                            name=f"{inst.name}-sw{uid[0]}",
                            opcode="NoOp",
                            engine=inst.engine,
                            debug=inst.debug,
                            ins=[],
                            outs=[],
                        )
                        nop.sync_info = mybir.SyncInfo(
                            on_wait=[w] if w else [], on_update=[u] if u else []
                        )
                        return nop

                    pre = [mknop(w=w) for w in ow[:-1]]
                    post = [mknop(u=u) for u in ou[1:]]
                    inst.sync_info = mybir.SyncInfo(
                        on_wait=ow[-1:], on_update=ou[:1]
                    )
                    changed = True
                out.extend(pre)
                out.append(inst)
                out.extend(post)
            if changed:
                bb.instructions = out


B, C, F, S = 1024, 3, 16, 28
P_OUT = 3
N_CORES = 8
BL = B // N_CORES  # 128 batch rows per core
NCLS = 5  # position classes per spatial axis
NJ = P_OUT * NCLS * NCLS * NCLS  # 375 distinct columns
SLOT_CD = [2, 0, 1, 3, 4]  # cd class per wexp slot (interior first)
F32 = mybir.dt.float32


def _conv3d_same(x, w):
    """x (B,Ci,D,H,W), w (Co,Ci,3,3,3) -> (B,Co,D,H,W), SAME padding."""
    Bp, Ci, D, H, W = x.shape
    xp = np.pad(x, ((0, 0), (0, 0), (1, 1), (1, 1), (1, 1)))
    out = np.zeros((Bp, w.shape[0], D, H, W), x.dtype)
    for kd in range(3):
        for kh in range(3):
            for kw in range(3):
                out += np.einsum(
                    "oc,bcdhw->bodhw",
                    w[:, :, kd, kh, kw],
                    xp[:, :, kd : kd + D, kh : kh + H, kw : kw + W],
                )
    return out


def _fold_weights(w1, b1, w2, b2):
    """Return W_aug (4, 375) float32: rows 0..2 = linear response to e_c at the
    5x5x5 class representatives, row 3 = constant term.  Column order is
    (p, slot, ch, cw) with slot = SLOT_CD order on the cd axis."""
    probe = np.zeros((4, C), np.float64)
    probe[:3] = np.eye(C)
    vp = np.broadcast_to(probe[:, :, None, None, None], (4, C, F, S, S)).astype(
        np.float64
    )
    y = _conv3d_same(vp, w1.astype(np.float64))
    y += b1.astype(np.float64)[None, :, None, None, None]
    y = _conv3d_same(y, w2.astype(np.float64))
    y += b2.astype(np.float64)[None, :, None, None, None]
    k0 = y[3]  # (3,16,28,28) constant part
    m = y[:3] - k0[None]  # (3,3,16,28,28) linear part

    dr = [0, 1, 2, F - 2, F - 1]
    hr = [0, 1, 2, S - 2, S - 1]
    mreps = m[:, :, dr][:, :, :, hr][:, :, :, :, hr]  # (3, 3, 5, 5, 5)
    kreps = k0[:, dr][:, :, hr][:, :, :, hr]  # (3, 5, 5, 5)
    mreps = mreps[:, :, SLOT_CD]  # cd axis -> slot order
    kreps = kreps[:, SLOT_CD]
    w_aug = np.empty((4, NJ), np.float64)
    w_aug[:3] = mreps.reshape(3, NJ)
    w_aug[3] = kreps.reshape(NJ)
    return np.ascontiguousarray(w_aug.astype(np.float32))


def _build_bass():
    nc = bass.Bass()
    # packed input: cols [0:BL] = x_aug^T (4,128), cols [BL:] = W_aug (4,375)
    xw = nc.dram_tensor("xw", [4, BL + NJ], F32, kind="ExternalInput")
    out = nc.dram_tensor("out", [BL, P_OUT, F, S, S], F32, kind="ExternalOutput")
    out_v = out[:].rearrange("b p d h w -> b p d (h w)")  # (128, 3, 16, 784)

    J1 = NCLS * NCLS  # 25: p0 interior (slot 0) block

    with TileContext(nc) as tc:
        with (
            tc.tile_pool(name="pool", bufs=1) as pool,
            tc.tile_pool(name="psum", bufs=1, space="PSUM") as psum_pool,
        ):
            xw_sb = pool.tile([4, BL + NJ], F32)
            nc.sync.dma_start(out=xw_sb[:], in_=xw[:])

            ps_a = psum_pool.tile([BL, J1], F32)
            ps_b = psum_pool.tile([BL, NJ - J1], F32)
            nc.tensor.matmul(
                ps_a[:], xw_sb[:, :BL], xw_sb[:, BL : BL + J1],
                start=True, stop=True,
            )
            nc.tensor.matmul(
                ps_b[:], xw_sb[:, :BL], xw_sb[:, BL + J1 :],
                start=True, stop=True,
            )

            # wexp[b, p, slot, ch, w]: w-axis 5 -> 28
            # dexp[b, p, s, h, w]: 8 d-slabs [cd0, cd1, I, I, I, I, cd3, cd4];
            # interior frames 6:14 re-read slabs 2:6 (plain repeated DMA reads)
            NSL = 8
            wexp = pool.tile([BL, P_OUT, NCLS, NCLS, S], F32)
            dexp = pool.tile([BL, P_OUT, NSL, S, S], F32)
            dv = dexp[:].rearrange("b p s h w -> b p s (h w)")  # (128, 3, 8, 784)

            def wexp_do(dst, src):
                """dst (BL, g, 5, 28) <- src (BL, g, 5, 5) w-expansion."""
                g = dst.shape[1]
                nc.vector.tensor_copy(
                    out=dst[:, :, :, 2 : S - 2],
                    in_=src[:, :, :, 2:3].to_broadcast((BL, g, NCLS, S - 4)),
                )
                nc.vector.tensor_copy(out=dst[:, :, :, 0:2], in_=src[:, :, :, 0:2])
                nc.vector.tensor_copy(
                    out=dst[:, :, :, S - 2 : S], in_=src[:, :, :, 3:5]
                )

            def hexp_interior(p, dsl, nf):
                """dexp[:, p, dsl] (nf frames) <- wexp[:, p, 0] h-expansion."""
                dst = dexp[:, p, dsl]
                src = wexp[:, p, 0:1]  # (BL, 1, 5, 28)
                nc.vector.tensor_copy(
                    out=dst[:, :, 2 : S - 2, :],
                    in_=src[:, :, 2:3, :].to_broadcast((BL, nf, S - 4, S)),
                )
                nc.vector.tensor_copy(
                    out=dst[:, :, 0:2, :],
                    in_=src[:, :, 0:2, :].to_broadcast((BL, nf, 2, S)),
                )
                nc.vector.tensor_copy(
                    out=dst[:, :, S - 2 : S, :],
                    in_=src[:, :, 3:5, :].to_broadcast((BL, nf, 2, S)),
                )

            def hexp_edges(p, dsl, ssl):
                """dexp[:, p, dsl] (2 frames) <- wexp[:, p, ssl] (2 slots)."""
                dst = dexp[:, p, dsl]
                src = wexp[:, p, ssl]  # (BL, 2, 5, 28)
                nc.vector.tensor_copy(
                    out=dst[:, :, 2 : S - 2, :],
                    in_=src[:, :, 2:3, :].to_broadcast((BL, 2, S - 4, S)),
                )
                nc.vector.tensor_copy(out=dst[:, :, 0:2, :], in_=src[:, :, 0:2, :])
                nc.vector.tensor_copy(
                    out=dst[:, :, S - 2 : S, :], in_=src[:, :, 3:5, :]
                )

            # ---- p0 interior: shortest path to the first output DMA ----
            wexp_do(
                wexp[:, 0, 0:1],
                ps_a[:].rearrange("b (g ch cw) -> b g ch cw", g=1, ch=NCLS),
            )
            hexp_interior(0, slice(2, 4), 2)
            nc.sync.dma_start(out=out_v[:, 0, 2:4, :], in_=dv[:, 0, 2:4, :])
            hexp_interior(0, slice(4, 6), 2)
            nc.sync.dma_start(out=out_v[:, 0, 4:6, :], in_=dv[:, 0, 4:6, :])
            # frames 6:14 re-read the already-expanded slabs 2:6 (no new copies)
            nc.sync.dma_start(out=out_v[:, 0, 6:10, :], in_=dv[:, 0, 2:6, :])
            nc.sync.dma_start(out=out_v[:, 0, 10:14, :], in_=dv[:, 0, 2:6, :])

            # ---- remaining w-expansions (reading PSUM B directly) ----
            pb = ps_b[:].rearrange("b (g ch cw) -> b g ch cw", g=14, ch=NCLS)
            wexp_do(wexp[:, 0, 1:NCLS], pb[:, 0:4])
            wexp_do(
                wexp[:, 1:P_OUT].rearrange("b p n c w -> b (p n) c w"), pb[:, 4:14]
            )

            # ---- p1 / p2 interiors: 3 DMAs each, 12544B descriptors ----
            for p in (1, 2):
                hexp_interior(p, slice(2, 6), 4)
                nc.sync.dma_start(out=out_v[:, p, 2:6, :], in_=dv[:, p, 2:6, :])
                nc.sync.dma_start(out=out_v[:, p, 6:10, :], in_=dv[:, p, 2:6, :])
                nc.sync.dma_start(out=out_v[:, p, 10:14, :], in_=dv[:, p, 2:6, :])

            # ---- edge slabs, then 2 merged cross-p edge DMAs (2.41 MB each) --
            for p in range(P_OUT):
                hexp_edges(p, slice(0, 2), slice(1, 3))
            nc.sync.dma_start(out=out_v[:, :, 0:2, :], in_=dv[:, :, 0:2, :])
            for p in range(P_OUT):
                hexp_edges(p, slice(6, 8), slice(3, 5))
            nc.sync.dma_start(
                out=out_v[:, :, F - 2 : F, :], in_=dv[:, :, 6:8, :]
            )
    _split_multi_waits(nc)
    return nc


_CACHE = {}


def kernel(x, w1, b1, w2, b2):
    x = np.ascontiguousarray(np.asarray(x, np.float32))
    w_aug = _fold_weights(
        np.asarray(w1, np.float64),
        np.asarray(b1, np.float64),
        np.asarray(w2, np.float64),
        np.asarray(b2, np.float64),
    )
    if "nc" not in _CACHE:
        _CACHE["nc"] = _build_bass()
    nc = _CACHE["nc"]

    # shard batch across cores; packed (4, 128+375): x_aug^T | W_aug
    in_maps = []
    for i in range(N_CORES):
        xs = x[i * BL : (i + 1) * BL]  # (128, 3)
        xa = np.concatenate([xs, np.ones((BL, 1), np.float32)], axis=1)  # (128,4)
        in_maps.append(
            {"xw": np.ascontiguousarray(np.concatenate([xa.T, w_aug], axis=1))}
        )
    res = run_bass_kernel_spmd(nc, in_maps, core_ids=list(range(N_CORES)))
    _CACHE["last_results"] = res  # exec_time_ns etc. when BASS_TRACE=1
    return np.concatenate([r["out"] for r in res.results], axis=0)
